# revision 46
# baseline (speedup 1.0000x reference)
"""Bass/Tile TRN2 kernel for nn_AttentionHead (B=64, N=1024, d=512), 8-core data parallel.

Math (per batch):
    proj  = x @ W1 + b1                      [N, 2d]
    S     = proj @ relu(proj).T / sqrt(2d)   [N, N]
    P     = softmax(S, axis=-1)
    F     = P @ proj                         [N, 2d]
    out   = relu(F @ W2 + b2)                [N, d]

Kernel dataflow (transposed-score formulation, value path folded through W12):
    xT    = x.T (DMA transpose)                                 [d, N]
    projT = W1.T @ xT + b1; keyT = relu(projT)                  [2d, N]
    St[m,n] = sum_e keyT[e,m] projT[e,n];  Et = exp(St * scale) [m, n]
    r[n]  = sum_m Et[m,n]            (ones-column matmul)
    V2[m,t] = sum_d x[m,d] W12[d,t],  W12 = W1 @ W2 (host)      [m, t]
    Z[n,t]  = sum_m Et[m,n] V2[m,t]
    out   = relu(Z[n,t]/r[n] + c[t]),  c = b1 @ W2 + b2 (host)
(F@W2 = P@(x@W1W2) + (b1@W2); the bias-row contribution r[n]*c[t] is applied
 post 1/r normalization.)
All matmul operands bf16 (fp32 PSUM accumulate). Loops are ordered so each
stationary (lhsT) tile serves the two 512-wide free-dim chunks back-to-back;
a post-compile pass (_dedup_ldweights) elides the repeated LDWEIGHTS.
"""

import numpy as np

B, N, D = 64, 1024, 512
E = 2 * D
NCORES = 8
BPC = B // NCORES
P = 128
MG = N // P  # 8 token groups
DG = D // P  # 4 d groups
EG = E // P  # 8 e groups
NJ = N // 512  # 2 free-dim chunks
SCALE = float(1.0 / np.sqrt(2.0 * D))

_CACHE = {}


def _dedup_ldweights(nc):
    """Delete redundant InstLdweights: consecutive PE weight-loads of the same
    SBUF region keep the PE array's stationary operand, so the repeat load is a
    no-op costing ~107ns. Only sync-free LDWs are removed (waits/updates were
    already hoisted by bacc's move_matmul_waits_to_ldweights)."""
    import concourse.mybir as mybir

    removed = 0
    for bb in nc.m.functions[0].blocks:
        last_key = None
        keep = []
        for inst in bb.instructions:
            if str(getattr(inst, "engine", "")) != "EngineType.PE":
                keep.append(inst)
                continue
            if isinstance(inst, mybir.InstLdweights):
                ap = inst.ins[0]
                key = (
                    getattr(ap, "memref", None),
                    getattr(ap, "offset", None),
                    str(getattr(ap, "ap", None)),
                    str(getattr(ap, "dtype", None)),
                    str(getattr(inst, "tile_position", None)),
                    str(getattr(inst, "is_transpose", None)),
                )
                si = inst.sync_info
                sync_free = si is None or (not si.on_wait and not si.on_update)
                if key == last_key and sync_free:
                    removed += 1
                    continue
                last_key = key
            keep.append(inst)
        bb.instructions[:] = keep
    return removed


def _build(bpc=BPC):
    import concourse.mybir as mybir
    import concourse.tile as tile
    from concourse import bacc
    from contextlib import ExitStack

    BF = mybir.dt.bfloat16
    F8 = mybir.dt.float8e4
    F32 = mybir.dt.float32
    AF = mybir.ActivationFunctionType
    ALU = mybir.AluOpType
    DR = mybir.MatmulPerfMode.DoubleRow

    nc = bacc.Bacc("TRN2", target_bir_lowering=False, debug=False, num_devices=NCORES)
    x_d = nc.dram_tensor("x", [bpc, N, D], F32, kind="ExternalInput").ap()
    w1_d = nc.dram_tensor("W1", [D, E], BF, kind="ExternalInput").ap()  # bf16 (host)
    b1_d = nc.dram_tensor("bias1", [E], F32, kind="ExternalInput").ap()
    w12_d = nc.dram_tensor("W12", [D, D], BF, kind="ExternalInput").ap()  # W1@W2 bf16 (host)
    c_d = nc.dram_tensor("c", [D], BF, kind="ExternalInput").ap()  # b1@W2 + b2 (host)
    out_d = nc.dram_tensor("out", [bpc, N, D], F32, kind="ExternalOutput").ap()

    with tile.TileContext(nc) as tc, ExitStack() as ctx:
        stage = ctx.enter_context(tc.tile_pool(name="stage", bufs=2))
        consts = ctx.enter_context(tc.tile_pool(name="consts", bufs=1))
        xbf_p = ctx.enter_context(tc.tile_pool(name="xbf", bufs=2))
        xt_p = ctx.enter_context(tc.tile_pool(name="xt", bufs=2))
        projT_p = ctx.enter_context(tc.tile_pool(name="projT", bufs=2))
        keyT_p = ctx.enter_context(tc.tile_pool(name="keyT", bufs=2))
        e_p = ctx.enter_context(tc.tile_pool(name="Et", bufs=2))
        v2_p = ctx.enter_context(tc.tile_pool(name="V2", bufs=2))
        outp = ctx.enter_context(tc.tile_pool(name="outp", bufs=2))
        small = ctx.enter_context(tc.tile_pool(name="small", bufs=2))
        dram = ctx.enter_context(tc.tile_pool(name="dram", bufs=2, space="DRAM"))
        ps = ctx.enter_context(tc.tile_pool(name="ps", bufs=6, space="PSUM"))
        psC = ctx.enter_context(tc.tile_pool(name="psC", bufs=2, space="PSUM"))

        # ---------------- x prep: load + cast + xbar transpose ----------------
        # PREP(b) is emitted during body b-1 (software pipelining) so the
        # vector casts sit ahead of body b-1's epilogue in the vector queue
        # and xT(b) is ready the moment fc1(b) can issue on PE.
        def prep(b):
            x_st = stage.tile([P, MG, D], F32, tag="stage")
            x_bf = xbf_p.tile([P, MG, D], BF)
            xT = xt_p.tile([P, DG, N], BF)
            if b == 0:
                # startup latency chain: alternate load queues (sync/gpsimd —
                # safe only before any out-stores exist) so each transpose can
                # chase its chunk instead of queueing behind the whole load
                for mg in range(MG):
                    ld = nc.sync if mg % 2 == 0 else nc.gpsimd
                    ld.dma_start(
                        out=x_st[:, mg, :], in_=x_d[0][mg * P : (mg + 1) * P, :]
                    )
                    nc.vector.tensor_copy(x_bf[:, mg, :], x_st[:, mg, :])
                    nc.sync.dma_start(
                        out=xT[:, :, mg * P : (mg + 1) * P],
                        in_=x_bf[:, mg, :],
                        transpose=True,
                    )
                return xT
            nc.sync.dma_start(
                out=x_st[:], in_=x_d[b].rearrange("(g p) d -> p g d", p=P)
            )
            for mg in range(MG):
                # gpsimd casts: keeps them out of the in-order vector queue,
                # where they'd transitively stall PE on this batch's drains
                nc.gpsimd.tensor_copy(x_bf[:, mg, :], x_st[:, mg, :])
                nc.sync.dma_start(
                    out=xT[:, :, mg * P : (mg + 1) * P],
                    in_=x_bf[:, mg, :],
                    transpose=True,
                )
            return xT

        xT = prep(0)

        # ---------------- constants / weights ----------------
        # W1 arrives pre-scaled by 8 (host) so it sits in e4m3 normal range;
        # the x8 on projT/keyT (x64 on scores) is undone in the exp scale.
        w1_bf = consts.tile([P, DG, E], BF)
        nc.scalar.dma_start(out=w1_bf[:], in_=w1_d.rearrange("(dg p) e -> p dg e", p=P))

        w12_bf = consts.tile([P, DG, D], BF)
        nc.scalar.dma_start(
            out=w12_bf[:], in_=w12_d.rearrange("(dg p) t -> p dg t", p=P)
        )

        b1t = consts.tile([P, EG], F32)
        nc.scalar.dma_start(out=b1t[:], in_=b1_d.rearrange("(g p) -> p g", p=P))
        ones_sq = consts.tile([P, P], BF)
        nc.vector.memset(ones_sq[:], 1.0)

        # c = b1@W2 + b2 (host, bf16), broadcast to all partitions. Adding c[t]
        # to every V2 row makes the fc2 psum Z + r[n]*c[t] (since sum_m Et = r),
        # so the epilogue is just out = relu(psum/r).
        import concourse.bass as bass_mod

        c_bcast = consts.tile([P, D], BF)
        c_src = c_d.rearrange("(o t) -> o t", o=1)
        c_bcast_ap = bass_mod.AP(
            tensor=c_src.tensor,
            offset=c_src.offset,
            ap=[[0, P], c_src.ap[1]],
        )
        nc.scalar.dma_start(out=c_bcast[:], in_=c_bcast_ap)

        # ---------------- per-batch pipeline ----------------
        for b in range(bpc):
            # fc1 (bf16): projT = 8*(W1.T @ xT) + 8*b1 ; keyT = relu(projT)
            # (projT/keyT are stored fp8 for the DoubleRow scores matmul; fc1
            #  itself stays bf16 so the fp8 quantization error isn't compounded)
            projT = projT_p.tile([P, EG, N], F8)
            keyT = keyT_p.tile([P, EG, N], F8)
            for eg in range(EG):
                pf = [ps.tile([P, 512], F32, tag="ps", name=f"pf{eg}_{j}") for j in range(NJ)]
                for dg in range(DG):
                    for nj in range(NJ):
                        nc.tensor.matmul(
                            pf[nj][:],
                            w1_bf[:, dg, eg * P : (eg + 1) * P],
                            xT[:, dg, nj * 512 : (nj + 1) * 512],
                            start=(dg == 0), stop=(dg == DG - 1),
                        )
                for nj in range(NJ):
                    nsl = slice(nj * 512, (nj + 1) * 512)
                    nc.scalar.activation(
                        projT[:, eg, nsl], pf[nj][:], AF.Identity,
                        bias=b1t[:, eg : eg + 1], scale=1.0,
                    )
                    nc.vector.tensor_scalar(
                        out=keyT[:, eg, nsl], in0=pf[nj][:],
                        scalar1=b1t[:, eg : eg + 1], scalar2=0.0,
                        op0=ALU.add, op1=ALU.max,
                    )

            # next batch's x prep: emitted after fc1 so its vector casts run
            # after this batch's keyT drains; all DMAs overlap this compute
            if b < bpc - 1:
                xT_next = prep(b + 1)

            # V2[m,t] = sum_d x[m,d] W12[d,t]  (independent of scores; placed
            # here to cover the projT/keyT PSUM-drain latency on PE)
            V2 = v2_p.tile([P, MG, D], BF)
            for mg in range(MG):
                pv = ps.tile([P, 512], F32, tag="ps", name=f"pv{mg}")
                for dg in range(DG):
                    nc.tensor.matmul(
                        pv[:],
                        xT[:, dg, mg * P : (mg + 1) * P],
                        w12_bf[:, dg, :],
                        start=(dg == 0), stop=(dg == DG - 1),
                    )
                nc.vector.tensor_add(V2[:, mg, :], pv[:], c_bcast[:])

            # St[m,n] = 64*sum_e keyT[e,m]*projT[e,n];  Et = exp(St*SCALE/64)
            Et = e_p.tile([P, MG, N], BF)
            for mg in range(MG):
                pst = [ps.tile([P, 512], F32, tag="ps", name=f"pst{mg}_{j}") for j in range(NJ)]
                for egp in range(EG // 2):
                    for nj in range(NJ):
                        nc.tensor.matmul(
                            pst[nj][:],
                            keyT[:, 2 * egp : 2 * egp + 2, mg * P : (mg + 1) * P],
                            projT[:, 2 * egp : 2 * egp + 2, nj * 512 : (nj + 1) * 512],
                            start=(egp == 0), stop=(egp == EG // 2 - 1),
                            perf_mode=DR,
                        )
                for nj in range(NJ):
                    nc.scalar.activation(
                        Et[:, mg, nj * 512 : (nj + 1) * 512], pst[nj][:], AF.Exp,
                        bias=0.0, scale=SCALE / 64.0,
                    )

            # rowsum r[n] = sum_m Et[m,n] (all-ones stationary; any psum row =
            # sum); drains on scalar to keep the vector queue free.
            r_f32 = small.tile([1, N], F32)
            pr = [ps.tile([P, 512], F32, tag="ps", name=f"pr{j}") for j in range(NJ)]
            for mg in range(MG):
                for nj in range(NJ):
                    nc.tensor.matmul(
                        pr[nj][:], ones_sq[:], Et[:, mg, nj * 512 : (nj + 1) * 512],
                        start=(mg == 0), stop=(mg == MG - 1),
                    )
            for nj in range(NJ):
                nsl = slice(nj * 512, (nj + 1) * 512)
                nc.scalar.copy(r_f32[:, nsl], pr[nj][0:1, :])

            # 1/r in [n-partition, 1] layout (bounce through DRAM to transpose).
            # On the vector queue: keeps the sync queue free for the next
            # batch's x load + transposes.
            r_dram = dram.tile([N], F32)
            nc.scalar.dma_start(out=r_dram.rearrange("(o n) -> o n", o=1), in_=r_f32[:1, :])
            rT = small.tile([P, MG], F32)
            nc.scalar.dma_start(out=rT[:], in_=r_dram.rearrange("(j p) -> p j", p=P))
            rinv = small.tile([P, MG], F32)
            nc.vector.reciprocal(rinv[:], rT[:])

            # Z[n,t] = sum_m Et[m,n] V2[m,t] + r[n]*c[t];  out = relu(Z/r)
            o_t = outp.tile([P, MG, D], F32)
            for ng in range(MG):
                po = psC.tile([P, D], F32, tag="psC")
                for mg in range(MG):
                    nc.tensor.matmul(
                        po[:],
                        Et[:, mg, ng * P : (ng + 1) * P],
                        V2[:, mg, :],
                        start=(mg == 0), stop=(mg == MG - 1),
                    )
                osl = o_t[:, ng, :]
                nc.scalar.activation(
                    osl, po[:], AF.Relu, bias=0.0, scale=rinv[:, ng : ng + 1]
                )
                if b == bpc - 1:
                    eng = nc.gpsimd if ng % 2 == 0 else nc.sync
                    eng.dma_start(
                        out=out_d[b][ng * P : (ng + 1) * P, :], in_=osl
                    )
            if b < bpc - 1:
                nc.gpsimd.dma_start(
                    out=out_d[b].rearrange("(g p) t -> p g t", p=P), in_=o_t[:]
                )
                xT = xT_next

    nc.compile()
    _dedup_ldweights(nc)
    return nc


def get_nc(bpc=BPC):
    if bpc not in _CACHE:
        _CACHE[bpc] = _build(bpc)
    return _CACHE[bpc]


def make_in_maps(x, W1, bias1, W2, bias2):
    x = np.ascontiguousarray(x, dtype=np.float32)
    W1 = np.asarray(W1, dtype=np.float32)
    bias1 = np.asarray(bias1, dtype=np.float32)
    W2 = np.asarray(W2, dtype=np.float32)
    bias2 = np.asarray(bias2, dtype=np.float32)
    c = (bias1 @ W2 + bias2).astype(np.float32)
    W12 = (W1 @ W2).astype(np.float32)
    # logit path runs in fp8: pre-scale W1/b1 by 8 so W1 lands in e4m3
    # normal range; the kernel divides the scores by 64 in the exp scale.
    # W1/W12 ship as bf16 so the kernel needs no weight casts.
    import ml_dtypes

    W1s = (8.0 * W1).astype(ml_dtypes.bfloat16)
    b1s = (8.0 * bias1).astype(np.float32)
    W12 = W12.astype(ml_dtypes.bfloat16)
    c = c.astype(ml_dtypes.bfloat16)
    return [
        {
            "x": x[i * BPC : (i + 1) * BPC],
            "W1": W1s,
            "bias1": b1s,
            "W12": W12,
            "c": c,
        }
        for i in range(NCORES)
    ]


def kernel(x, W1, bias1, W2, bias2):
    from concourse.bass_utils import run_bass_kernel_spmd

    nc = get_nc()
    in_maps = make_in_maps(x, W1, bias1, W2, bias2)
    res = run_bass_kernel_spmd(nc, in_maps, list(range(NCORES)))
    return np.concatenate([res.results[i]["out"] for i in range(NCORES)], axis=0)


# revision 49
# speedup vs baseline: 1.0538x; 1.0538x over previous
"""Bass/Tile TRN2 kernel for nn_AttentionHead (B=64, N=1024, d=512), 8-core data parallel.

Math (per batch):
    proj  = x @ W1 + b1                      [N, 2d]
    S     = proj @ relu(proj).T / sqrt(2d)   [N, N]
    P     = softmax(S, axis=-1)
    F     = P @ proj                         [N, 2d]
    out   = relu(F @ W2 + b2)                [N, d]

Kernel dataflow (transposed-score formulation, value path folded through W12):
    xT    = x.T (DMA transpose)                                 [d, N]
    projT = W1.T @ xT + b1; keyT = relu(projT)                  [2d, N]
    St[m,n] = sum_e keyT[e,m] projT[e,n];  Et = exp(St * scale) [m, n]
    r[n]  = sum_m Et[m,n]            (ones-column matmul)
    V2[m,t] = sum_d x[m,d] W12[d,t],  W12 = W1 @ W2 (host)      [m, t]
    Z[n,t]  = sum_m Et[m,n] V2[m,t]
    out   = relu(Z[n,t]/r[n] + c[t]),  c = b1 @ W2 + b2 (host)
(F@W2 = P@(x@W1W2) + (b1@W2); the bias-row contribution r[n]*c[t] is applied
 post 1/r normalization.)
All matmul operands bf16 (fp32 PSUM accumulate). Loops are ordered so each
stationary (lhsT) tile serves the two 512-wide free-dim chunks back-to-back;
a post-compile pass (_dedup_ldweights) elides the repeated LDWEIGHTS.
"""

import numpy as np

B, N, D = 64, 1024, 512
E = 2 * D
NCORES = 8
BPC = B // NCORES
P = 128
MG = N // P  # 8 token groups
DG = D // P  # 4 d groups
EG = E // P  # 8 e groups
NJ = N // 512  # 2 free-dim chunks
SCALE = float(1.0 / np.sqrt(2.0 * D))

_CACHE = {}


def _dedup_ldweights(nc):
    """Delete redundant InstLdweights: consecutive PE weight-loads of the same
    SBUF region keep the PE array's stationary operand, so the repeat load is a
    no-op costing ~107ns. Only sync-free LDWs are removed (waits/updates were
    already hoisted by bacc's move_matmul_waits_to_ldweights)."""
    import concourse.mybir as mybir

    removed = 0
    for bb in nc.m.functions[0].blocks:
        last_key = None
        keep = []
        for inst in bb.instructions:
            if str(getattr(inst, "engine", "")) != "EngineType.PE":
                keep.append(inst)
                continue
            if isinstance(inst, mybir.InstLdweights):
                ap = inst.ins[0]
                key = (
                    getattr(ap, "memref", None),
                    getattr(ap, "offset", None),
                    str(getattr(ap, "ap", None)),
                    str(getattr(ap, "dtype", None)),
                    str(getattr(inst, "tile_position", None)),
                    str(getattr(inst, "is_transpose", None)),
                )
                si = inst.sync_info
                sync_free = si is None or (not si.on_wait and not si.on_update)
                if key == last_key and sync_free:
                    removed += 1
                    continue
                last_key = key
            keep.append(inst)
        bb.instructions[:] = keep
    return removed


def _build(bpc=BPC):
    import concourse.mybir as mybir
    import concourse.tile as tile
    from concourse import bacc
    from contextlib import ExitStack

    BF = mybir.dt.bfloat16
    F8 = mybir.dt.float8e4
    F32 = mybir.dt.float32
    AF = mybir.ActivationFunctionType
    ALU = mybir.AluOpType
    DR = mybir.MatmulPerfMode.DoubleRow

    nc = bacc.Bacc("TRN2", target_bir_lowering=False, debug=False, num_devices=NCORES)
    x_d = nc.dram_tensor("x", [bpc, N, D], F32, kind="ExternalInput").ap()
    w1_d = nc.dram_tensor("W1", [D, E], BF, kind="ExternalInput").ap()  # bf16 (host)
    b1_d = nc.dram_tensor("bias1", [E], F32, kind="ExternalInput").ap()
    w12_d = nc.dram_tensor("W12", [D, D], BF, kind="ExternalInput").ap()  # W1@W2 bf16 (host)
    c_d = nc.dram_tensor("c", [D], BF, kind="ExternalInput").ap()  # b1@W2 + b2 (host)
    out_d = nc.dram_tensor("out", [bpc, N, D], F32, kind="ExternalOutput").ap()

    with tile.TileContext(nc) as tc, ExitStack() as ctx:
        stage = ctx.enter_context(tc.tile_pool(name="stage", bufs=2))
        consts = ctx.enter_context(tc.tile_pool(name="consts", bufs=1))
        xbf_p = ctx.enter_context(tc.tile_pool(name="xbf", bufs=2))
        xt_p = ctx.enter_context(tc.tile_pool(name="xt", bufs=2))
        projT_p = ctx.enter_context(tc.tile_pool(name="projT", bufs=2))
        keyT_p = ctx.enter_context(tc.tile_pool(name="keyT", bufs=2))
        e_p = ctx.enter_context(tc.tile_pool(name="Et", bufs=2))
        v2_p = ctx.enter_context(tc.tile_pool(name="V2", bufs=2))
        outp = ctx.enter_context(tc.tile_pool(name="outp", bufs=2))
        small = ctx.enter_context(tc.tile_pool(name="small", bufs=2))
        dram = ctx.enter_context(tc.tile_pool(name="dram", bufs=2, space="DRAM"))
        ps = ctx.enter_context(tc.tile_pool(name="ps", bufs=6, space="PSUM"))
        psC = ctx.enter_context(tc.tile_pool(name="psC", bufs=2, space="PSUM"))

        # ---------------- x prep: load + cast + xbar transpose ----------------
        # PREP(b) is emitted during body b-1 (software pipelining) so the
        # vector casts sit ahead of body b-1's epilogue in the vector queue
        # and xT(b) is ready the moment fc1(b) can issue on PE.
        def prep(b):
            x_st = stage.tile([P, MG, D], F32, tag="stage")
            x_bf = xbf_p.tile([P, MG, D], BF)
            xT = xt_p.tile([P, DG, N], BF)
            if b == 0:
                # startup latency chain: alternate load queues (sync/gpsimd —
                # safe only before any out-stores exist) so each transpose can
                # chase its chunk instead of queueing behind the whole load
                for mg in range(MG):
                    ld = nc.sync if mg % 2 == 0 else nc.gpsimd
                    ld.dma_start(
                        out=x_st[:, mg, :], in_=x_d[0][mg * P : (mg + 1) * P, :]
                    )
                    nc.vector.tensor_copy(x_bf[:, mg, :], x_st[:, mg, :])
                    nc.sync.dma_start(
                        out=xT[:, :, mg * P : (mg + 1) * P],
                        in_=x_bf[:, mg, :],
                        transpose=True,
                    )
                return xT
            nc.sync.dma_start(
                out=x_st[:], in_=x_d[b].rearrange("(g p) d -> p g d", p=P)
            )
            for mg in range(MG):
                nc.vector.tensor_copy(x_bf[:, mg, :], x_st[:, mg, :])
                nc.sync.dma_start(
                    out=xT[:, :, mg * P : (mg + 1) * P],
                    in_=x_bf[:, mg, :],
                    transpose=True,
                )
            return xT

        xT = prep(0)

        # ---------------- constants / weights ----------------
        # W1 arrives pre-scaled by 8 (host) so it sits in e4m3 normal range;
        # the x8 on projT/keyT (x64 on scores) is undone in the exp scale.
        w1_bf = consts.tile([P, DG, E], BF)
        nc.scalar.dma_start(out=w1_bf[:], in_=w1_d.rearrange("(dg p) e -> p dg e", p=P))

        w12_bf = consts.tile([P, DG, D], BF)
        nc.scalar.dma_start(
            out=w12_bf[:], in_=w12_d.rearrange("(dg p) t -> p dg t", p=P)
        )

        b1t = consts.tile([P, EG], F32)
        nc.scalar.dma_start(out=b1t[:], in_=b1_d.rearrange("(g p) -> p g", p=P))
        ones_sq = consts.tile([P, P], BF)
        nc.vector.memset(ones_sq[:], 1.0)

        # c = b1@W2 + b2 (host, bf16), broadcast to all partitions. Adding c[t]
        # to every V2 row makes the fc2 psum Z + r[n]*c[t] (since sum_m Et = r),
        # so the epilogue is just out = relu(psum/r).
        import concourse.bass as bass_mod

        c_bcast = consts.tile([P, D], BF)
        c_src = c_d.rearrange("(o t) -> o t", o=1)
        c_bcast_ap = bass_mod.AP(
            tensor=c_src.tensor,
            offset=c_src.offset,
            ap=[[0, P], c_src.ap[1]],
        )
        nc.scalar.dma_start(out=c_bcast[:], in_=c_bcast_ap)

        # ---------------- per-batch pipeline ----------------
        for b in range(bpc):
            # fc1 (bf16): projT = 8*(W1.T @ xT) + 8*b1 ; keyT = relu(projT)
            # (projT/keyT are stored fp8 for the DoubleRow scores matmul; fc1
            #  itself stays bf16 so the fp8 quantization error isn't compounded)
            projT = projT_p.tile([P, EG, N], F8)
            keyT = keyT_p.tile([P, EG, N], F8)
            for eg in range(EG):
                pf = [ps.tile([P, 512], F32, tag="ps", name=f"pf{eg}_{j}") for j in range(NJ)]
                for dg in range(DG):
                    for nj in range(NJ):
                        nc.tensor.matmul(
                            pf[nj][:],
                            w1_bf[:, dg, eg * P : (eg + 1) * P],
                            xT[:, dg, nj * 512 : (nj + 1) * 512],
                            start=(dg == 0), stop=(dg == DG - 1),
                        )
                for nj in range(NJ):
                    nsl = slice(nj * 512, (nj + 1) * 512)
                    nc.scalar.activation(
                        projT[:, eg, nsl], pf[nj][:], AF.Identity,
                        bias=b1t[:, eg : eg + 1], scale=1.0,
                    )
                    nc.vector.tensor_scalar(
                        out=keyT[:, eg, nsl], in0=pf[nj][:],
                        scalar1=b1t[:, eg : eg + 1], scalar2=0.0,
                        op0=ALU.add, op1=ALU.max,
                    )

            # next batch's x prep: emitted after fc1 so its vector casts run
            # after this batch's keyT drains; all DMAs overlap this compute
            if b < bpc - 1:
                xT_next = prep(b + 1)

            # V2[m,t] = sum_d x[m,d] W12[d,t]  (independent of scores). First
            # half covers the projT/keyT PSUM-drain latency on PE; second half
            # is emitted after scores to cover the last exp before rowsum.
            V2 = v2_p.tile([P, MG, D], BF)

            def v2_half(lo, hi):
                for mg in range(lo, hi):
                    pv = ps.tile([P, 512], F32, tag="ps", name=f"pv{mg}")
                    for dg in range(DG):
                        nc.tensor.matmul(
                            pv[:],
                            xT[:, dg, mg * P : (mg + 1) * P],
                            w12_bf[:, dg, :],
                            start=(dg == 0), stop=(dg == DG - 1),
                        )
                    nc.vector.tensor_add(V2[:, mg, :], pv[:], c_bcast[:])

            v2_half(0, MG // 2)

            # St[m,n] = 64*sum_e keyT[e,m]*projT[e,n];  Et = exp(St*SCALE/64)
            Et = e_p.tile([P, MG, N], BF)
            for mg in range(MG):
                pst = [ps.tile([P, 512], F32, tag="ps", name=f"pst{mg}_{j}") for j in range(NJ)]
                for egp in range(EG // 2):
                    for nj in range(NJ):
                        nc.tensor.matmul(
                            pst[nj][:],
                            keyT[:, 2 * egp : 2 * egp + 2, mg * P : (mg + 1) * P],
                            projT[:, 2 * egp : 2 * egp + 2, nj * 512 : (nj + 1) * 512],
                            start=(egp == 0), stop=(egp == EG // 2 - 1),
                            perf_mode=DR,
                        )
                for nj in range(NJ):
                    nc.scalar.activation(
                        Et[:, mg, nj * 512 : (nj + 1) * 512], pst[nj][:], AF.Exp,
                        bias=0.0, scale=SCALE / 64.0,
                    )

            v2_half(MG // 2, MG)

            # rowsum r[n] = sum_m Et[m,n] (all-ones stationary; any psum row =
            # sum); drains on scalar to keep the vector queue free.
            r_f32 = small.tile([1, N], F32)
            pr = [ps.tile([P, 512], F32, tag="ps", name=f"pr{j}") for j in range(NJ)]
            for mg in range(MG):
                for nj in range(NJ):
                    nc.tensor.matmul(
                        pr[nj][:], ones_sq[:], Et[:, mg, nj * 512 : (nj + 1) * 512],
                        start=(mg == 0), stop=(mg == MG - 1),
                    )
            for nj in range(NJ):
                nsl = slice(nj * 512, (nj + 1) * 512)
                nc.scalar.copy(r_f32[:, nsl], pr[nj][0:1, :])

            # 1/r in [n-partition, 1] layout (bounce through DRAM to transpose).
            # On the vector queue: keeps the sync queue free for the next
            # batch's x load + transposes.
            r_dram = dram.tile([N], F32)
            nc.scalar.dma_start(out=r_dram.rearrange("(o n) -> o n", o=1), in_=r_f32[:1, :])
            rT = small.tile([P, MG], F32)
            nc.scalar.dma_start(out=rT[:], in_=r_dram.rearrange("(j p) -> p j", p=P))
            rinv = small.tile([P, MG], F32)
            nc.vector.reciprocal(rinv[:], rT[:])

            # Z[n,t] = sum_m Et[m,n] V2[m,t] + r[n]*c[t];  out = relu(Z/r)
            o_t = outp.tile([P, MG, D], F32)
            for ng in range(MG):
                po = psC.tile([P, D], F32, tag="psC")
                for mg in range(MG):
                    nc.tensor.matmul(
                        po[:],
                        Et[:, mg, ng * P : (ng + 1) * P],
                        V2[:, mg, :],
                        start=(mg == 0), stop=(mg == MG - 1),
                    )
                osl = o_t[:, ng, :]
                nc.scalar.activation(
                    osl, po[:], AF.Relu, bias=0.0, scale=rinv[:, ng : ng + 1]
                )
                if b == bpc - 1:
                    eng = nc.gpsimd if ng % 2 == 0 else nc.sync
                    eng.dma_start(
                        out=out_d[b][ng * P : (ng + 1) * P, :], in_=osl
                    )
            if b < bpc - 1:
                nc.gpsimd.dma_start(
                    out=out_d[b].rearrange("(g p) t -> p g t", p=P), in_=o_t[:]
                )
                xT = xT_next

    nc.compile()
    _dedup_ldweights(nc)
    return nc


def get_nc(bpc=BPC):
    if bpc not in _CACHE:
        _CACHE[bpc] = _build(bpc)
    return _CACHE[bpc]


def make_in_maps(x, W1, bias1, W2, bias2):
    x = np.ascontiguousarray(x, dtype=np.float32)
    W1 = np.asarray(W1, dtype=np.float32)
    bias1 = np.asarray(bias1, dtype=np.float32)
    W2 = np.asarray(W2, dtype=np.float32)
    bias2 = np.asarray(bias2, dtype=np.float32)
    c = (bias1 @ W2 + bias2).astype(np.float32)
    W12 = (W1 @ W2).astype(np.float32)
    # logit path runs in fp8: pre-scale W1/b1 by 8 so W1 lands in e4m3
    # normal range; the kernel divides the scores by 64 in the exp scale.
    # W1/W12 ship as bf16 so the kernel needs no weight casts.
    import ml_dtypes

    W1s = (8.0 * W1).astype(ml_dtypes.bfloat16)
    b1s = (8.0 * bias1).astype(np.float32)
    W12 = W12.astype(ml_dtypes.bfloat16)
    c = c.astype(ml_dtypes.bfloat16)
    return [
        {
            "x": x[i * BPC : (i + 1) * BPC],
            "W1": W1s,
            "bias1": b1s,
            "W12": W12,
            "c": c,
        }
        for i in range(NCORES)
    ]


def kernel(x, W1, bias1, W2, bias2):
    from concourse.bass_utils import run_bass_kernel_spmd

    nc = get_nc()
    in_maps = make_in_maps(x, W1, bias1, W2, bias2)
    res = run_bass_kernel_spmd(nc, in_maps, list(range(NCORES)))
    return np.concatenate([res.results[i]["out"] for i in range(NCORES)], axis=0)


# revision 51
# speedup vs baseline: 1.1290x; 1.0713x over previous
"""Bass/Tile TRN2 kernel for nn_AttentionHead (B=64, N=1024, d=512), 8-core data parallel.

Math (per batch):
    proj  = x @ W1 + b1                      [N, 2d]
    S     = proj @ relu(proj).T / sqrt(2d)   [N, N]
    P     = softmax(S, axis=-1)
    F     = P @ proj                         [N, 2d]
    out   = relu(F @ W2 + b2)                [N, d]

Kernel dataflow (transposed-score formulation, value path folded through W12):
    xT    = x.T (DMA transpose)                                 [d, N]
    projT = W1.T @ xT + b1; keyT = relu(projT)                  [2d, N]
    St[m,n] = sum_e keyT[e,m] projT[e,n];  Et = exp(St * scale) [m, n]
    r[n]  = sum_m Et[m,n]            (ones-column matmul)
    V2[m,t] = sum_d x[m,d] W12[d,t],  W12 = W1 @ W2 (host)      [m, t]
    Z[n,t]  = sum_m Et[m,n] V2[m,t]
    out   = relu(Z[n,t]/r[n] + c[t]),  c = b1 @ W2 + b2 (host)
(F@W2 = P@(x@W1W2) + (b1@W2); the bias-row contribution r[n]*c[t] is applied
 post 1/r normalization.)
All matmul operands bf16 (fp32 PSUM accumulate). Loops are ordered so each
stationary (lhsT) tile serves the two 512-wide free-dim chunks back-to-back;
a post-compile pass (_dedup_ldweights) elides the repeated LDWEIGHTS.
"""

import numpy as np

B, N, D = 64, 1024, 512
E = 2 * D
NCORES = 8
BPC = B // NCORES
P = 128
MG = N // P  # 8 token groups
DG = D // P  # 4 d groups
EG = E // P  # 8 e groups
NJ = N // 512  # 2 free-dim chunks
SCALE = float(1.0 / np.sqrt(2.0 * D))

_CACHE = {}


def _dedup_ldweights(nc):
    """Delete redundant InstLdweights: consecutive PE weight-loads of the same
    SBUF region keep the PE array's stationary operand, so the repeat load is a
    no-op costing ~107ns. Only sync-free LDWs are removed (waits/updates were
    already hoisted by bacc's move_matmul_waits_to_ldweights)."""
    import concourse.mybir as mybir

    removed = 0
    for bb in nc.m.functions[0].blocks:
        last_key = None
        keep = []
        for inst in bb.instructions:
            if str(getattr(inst, "engine", "")) != "EngineType.PE":
                keep.append(inst)
                continue
            if isinstance(inst, mybir.InstLdweights):
                ap = inst.ins[0]
                key = (
                    getattr(ap, "memref", None),
                    getattr(ap, "offset", None),
                    str(getattr(ap, "ap", None)),
                    str(getattr(ap, "dtype", None)),
                    str(getattr(inst, "tile_position", None)),
                    str(getattr(inst, "is_transpose", None)),
                )
                si = inst.sync_info
                sync_free = si is None or (not si.on_wait and not si.on_update)
                if key == last_key and sync_free:
                    removed += 1
                    continue
                last_key = key
            keep.append(inst)
        bb.instructions[:] = keep
    return removed


def _build(bpc=BPC):
    import concourse.mybir as mybir
    import concourse.tile as tile
    from concourse import bacc
    from contextlib import ExitStack

    BF = mybir.dt.bfloat16
    F8 = mybir.dt.float8e4
    F32 = mybir.dt.float32
    AF = mybir.ActivationFunctionType
    ALU = mybir.AluOpType
    DR = mybir.MatmulPerfMode.DoubleRow

    nc = bacc.Bacc("TRN2", target_bir_lowering=False, debug=False, num_devices=NCORES)
    x_d = nc.dram_tensor("x", [bpc, N, D], F32, kind="ExternalInput").ap()
    w1_d = nc.dram_tensor("W1", [D, E], BF, kind="ExternalInput").ap()  # bf16 (host)
    b1_d = nc.dram_tensor("bias1", [E], F32, kind="ExternalInput").ap()
    w12_d = nc.dram_tensor("W12", [D, D], BF, kind="ExternalInput").ap()  # W1@W2 bf16 (host)
    c_d = nc.dram_tensor("c", [D], BF, kind="ExternalInput").ap()  # b1@W2 + b2 (host)
    out_d = nc.dram_tensor("out", [bpc, N, D], F32, kind="ExternalOutput").ap()

    with tile.TileContext(nc) as tc, ExitStack() as ctx:
        stage = ctx.enter_context(tc.tile_pool(name="stage", bufs=2))
        consts = ctx.enter_context(tc.tile_pool(name="consts", bufs=1))
        xbf_p = ctx.enter_context(tc.tile_pool(name="xbf", bufs=2))
        xt_p = ctx.enter_context(tc.tile_pool(name="xt", bufs=2))
        projT_p = ctx.enter_context(tc.tile_pool(name="projT", bufs=2))
        keyT_p = ctx.enter_context(tc.tile_pool(name="keyT", bufs=2))
        e_p = ctx.enter_context(tc.tile_pool(name="Et", bufs=2))
        v2_p = ctx.enter_context(tc.tile_pool(name="V2", bufs=2))
        outp = ctx.enter_context(tc.tile_pool(name="outp", bufs=2))
        small = ctx.enter_context(tc.tile_pool(name="small", bufs=2))
        dram = ctx.enter_context(tc.tile_pool(name="dram", bufs=2, space="DRAM"))
        ps = ctx.enter_context(tc.tile_pool(name="ps", bufs=6, space="PSUM"))
        psC = ctx.enter_context(tc.tile_pool(name="psC", bufs=2, space="PSUM"))

        # ---------------- x prep: load + cast + xbar transpose ----------------
        # PREP(b) is emitted during body b-1 (software pipelining) so the
        # vector casts sit ahead of body b-1's epilogue in the vector queue
        # and xT(b) is ready the moment fc1(b) can issue on PE.
        def prep(b):
            x_st = stage.tile([P, MG, D], F32, tag="stage")
            x_bf = xbf_p.tile([P, MG, D], BF)
            xT = xt_p.tile([P, DG, N], BF)
            if b == 0:
                # startup latency chain: alternate load queues (sync/gpsimd —
                # safe only before any out-stores exist) so each transpose can
                # chase its chunk instead of queueing behind the whole load
                for mg in range(MG):
                    ld = nc.sync if mg % 2 == 0 else nc.gpsimd
                    ld.dma_start(
                        out=x_st[:, mg, :], in_=x_d[0][mg * P : (mg + 1) * P, :]
                    )
                    nc.vector.tensor_copy(x_bf[:, mg, :], x_st[:, mg, :])
                    nc.sync.dma_start(
                        out=xT[:, :, mg * P : (mg + 1) * P],
                        in_=x_bf[:, mg, :],
                        transpose=True,
                    )
                return xT
            nc.sync.dma_start(
                out=x_st[:], in_=x_d[b].rearrange("(g p) d -> p g d", p=P)
            )
            for mg in range(MG):
                nc.vector.tensor_copy(x_bf[:, mg, :], x_st[:, mg, :])
                nc.sync.dma_start(
                    out=xT[:, :, mg * P : (mg + 1) * P],
                    in_=x_bf[:, mg, :],
                    transpose=True,
                )
            return xT

        xT = prep(0)

        # ---------------- constants / weights ----------------
        # W1 arrives pre-scaled by 8 (host) so it sits in e4m3 normal range;
        # the x8 on projT/keyT (x64 on scores) is undone in the exp scale.
        w1_bf = consts.tile([P, DG, E], BF)
        nc.scalar.dma_start(out=w1_bf[:], in_=w1_d.rearrange("(dg p) e -> p dg e", p=P))

        w12_bf = consts.tile([P, DG, D], BF)
        nc.scalar.dma_start(
            out=w12_bf[:], in_=w12_d.rearrange("(dg p) t -> p dg t", p=P)
        )

        b1t = consts.tile([P, EG], F32)
        nc.scalar.dma_start(out=b1t[:], in_=b1_d.rearrange("(g p) -> p g", p=P))
        ones_sq = consts.tile([P, P], BF)
        nc.vector.memset(ones_sq[:], 1.0)

        # c = b1@W2 + b2 (host, bf16), broadcast to all partitions. Adding c[t]
        # to every V2 row makes the fc2 psum Z + r[n]*c[t] (since sum_m Et = r),
        # so the epilogue is just out = relu(psum/r).
        import concourse.bass as bass_mod

        c_bcast = consts.tile([P, D], BF)
        c_src = c_d.rearrange("(o t) -> o t", o=1)
        c_bcast_ap = bass_mod.AP(
            tensor=c_src.tensor,
            offset=c_src.offset,
            ap=[[0, P], c_src.ap[1]],
        )
        nc.scalar.dma_start(out=c_bcast[:], in_=c_bcast_ap)

        # ---------------- per-batch pipeline ----------------
        for b in range(bpc):
            # fc1 (bf16): projT = 8*(W1.T @ xT) + 8*b1 ; keyT = relu(projT)
            # (projT/keyT are stored fp8 for the DoubleRow scores matmul; fc1
            #  itself stays bf16 so the fp8 quantization error isn't compounded)
            projT = projT_p.tile([P, EG, N], F8)
            keyT = keyT_p.tile([P, EG, N], F8)
            for eg in range(EG):
                pf = [ps.tile([P, 512], F32, tag="ps", name=f"pf{eg}_{j}") for j in range(NJ)]
                for dg in range(DG):
                    for nj in range(NJ):
                        nc.tensor.matmul(
                            pf[nj][:],
                            w1_bf[:, dg, eg * P : (eg + 1) * P],
                            xT[:, dg, nj * 512 : (nj + 1) * 512],
                            start=(dg == 0), stop=(dg == DG - 1),
                        )
                for nj in range(NJ):
                    nsl = slice(nj * 512, (nj + 1) * 512)
                    nc.scalar.activation(
                        projT[:, eg, nsl], pf[nj][:], AF.Identity,
                        bias=b1t[:, eg : eg + 1], scale=1.0,
                    )
                    nc.vector.tensor_scalar(
                        out=keyT[:, eg, nsl], in0=pf[nj][:],
                        scalar1=b1t[:, eg : eg + 1], scalar2=0.0,
                        op0=ALU.add, op1=ALU.max,
                    )

            # next batch's x prep: emitted after fc1 so its vector casts run
            # after this batch's keyT drains; all DMAs overlap this compute
            if b < bpc - 1:
                xT_next = prep(b + 1)

            # V2[m,t] = sum_d x[m,d] W12[d,t]  (independent of scores). First
            # half covers the projT/keyT PSUM-drain latency on PE; second half
            # is emitted after scores to cover the last exp before rowsum.
            V2 = v2_p.tile([P, MG, D], BF)

            def v2_half(lo, hi):
                for mg in range(lo, hi):
                    pv = ps.tile([P, 512], F32, tag="ps", name=f"pv{mg}")
                    for dg in range(DG):
                        nc.tensor.matmul(
                            pv[:],
                            xT[:, dg, mg * P : (mg + 1) * P],
                            w12_bf[:, dg, :],
                            start=(dg == 0), stop=(dg == DG - 1),
                        )
                    nc.vector.tensor_add(V2[:, mg, :], pv[:], c_bcast[:])

            v2_half(0, MG)

            # St[m,n] = 64*sum_e keyT[e,m]*projT[e,n];  Et = exp(St*SCALE/64)
            Et = e_p.tile([P, MG, N], BF)
            for mg in range(MG):
                pst = [ps.tile([P, 512], F32, tag="ps", name=f"pst{mg}_{j}") for j in range(NJ)]
                for egp in range(EG // 2):
                    for nj in range(NJ):
                        nc.tensor.matmul(
                            pst[nj][:],
                            keyT[:, 2 * egp : 2 * egp + 2, mg * P : (mg + 1) * P],
                            projT[:, 2 * egp : 2 * egp + 2, nj * 512 : (nj + 1) * 512],
                            start=(egp == 0), stop=(egp == EG // 2 - 1),
                            perf_mode=DR,
                        )
                for nj in range(NJ):
                    nc.scalar.activation(
                        Et[:, mg, nj * 512 : (nj + 1) * 512], pst[nj][:], AF.Exp,
                        bias=0.0, scale=SCALE / 64.0,
                    )

            # rowsum r[n] = sum_m Et[m,n] (all-ones stationary; any psum row =
            # sum); drains on scalar to keep the vector queue free.
            r_f32 = small.tile([1, N], F32)
            pr = [ps.tile([P, 512], F32, tag="ps", name=f"pr{j}") for j in range(NJ)]
            for mg in range(MG):
                for nj in range(NJ):
                    nc.tensor.matmul(
                        pr[nj][:], ones_sq[:], Et[:, mg, nj * 512 : (nj + 1) * 512],
                        start=(mg == 0), stop=(mg == MG - 1),
                    )
            for nj in range(NJ):
                nsl = slice(nj * 512, (nj + 1) * 512)
                nc.scalar.copy(r_f32[:, nsl], pr[nj][0:1, :])

            # 1/r in [n-partition, 1] layout (bounce through DRAM to transpose).
            # On the vector queue: keeps the sync queue free for the next
            # batch's x load + transposes.
            r_dram = dram.tile([N], F32)
            nc.scalar.dma_start(out=r_dram.rearrange("(o n) -> o n", o=1), in_=r_f32[:1, :])
            rT = small.tile([P, MG], F32)
            nc.scalar.dma_start(out=rT[:], in_=r_dram.rearrange("(j p) -> p j", p=P))
            rinv = small.tile([P, MG], F32)
            nc.vector.reciprocal(rinv[:], rT[:])

            # Z[n,t] = sum_m Et[m,n] V2[m,t] + r[n]*c[t];  out = relu(Z/r)
            o_t = outp.tile([P, MG, D], F32)
            for ng in range(MG):
                po = psC.tile([P, D], F32, tag="psC")
                for mg in range(MG):
                    nc.tensor.matmul(
                        po[:],
                        Et[:, mg, ng * P : (ng + 1) * P],
                        V2[:, mg, :],
                        start=(mg == 0), stop=(mg == MG - 1),
                    )
                osl = o_t[:, ng, :]
                nc.scalar.activation(
                    osl, po[:], AF.Relu, bias=0.0, scale=rinv[:, ng : ng + 1]
                )
                if b == bpc - 1:
                    eng = nc.gpsimd if ng % 2 == 0 else nc.sync
                    eng.dma_start(
                        out=out_d[b][ng * P : (ng + 1) * P, :], in_=osl
                    )
            if b < bpc - 1:
                nc.gpsimd.dma_start(
                    out=out_d[b].rearrange("(g p) t -> p g t", p=P), in_=o_t[:]
                )
                xT = xT_next

    nc.compile()
    _dedup_ldweights(nc)
    return nc


def get_nc(bpc=BPC):
    if bpc not in _CACHE:
        _CACHE[bpc] = _build(bpc)
    return _CACHE[bpc]


def make_in_maps(x, W1, bias1, W2, bias2):
    x = np.ascontiguousarray(x, dtype=np.float32)
    W1 = np.asarray(W1, dtype=np.float32)
    bias1 = np.asarray(bias1, dtype=np.float32)
    W2 = np.asarray(W2, dtype=np.float32)
    bias2 = np.asarray(bias2, dtype=np.float32)
    c = (bias1 @ W2 + bias2).astype(np.float32)
    W12 = (W1 @ W2).astype(np.float32)
    # logit path runs in fp8: pre-scale W1/b1 by 8 so W1 lands in e4m3
    # normal range; the kernel divides the scores by 64 in the exp scale.
    # W1/W12 ship as bf16 so the kernel needs no weight casts.
    import ml_dtypes

    W1s = (8.0 * W1).astype(ml_dtypes.bfloat16)
    b1s = (8.0 * bias1).astype(np.float32)
    W12 = W12.astype(ml_dtypes.bfloat16)
    c = c.astype(ml_dtypes.bfloat16)
    return [
        {
            "x": x[i * BPC : (i + 1) * BPC],
            "W1": W1s,
            "bias1": b1s,
            "W12": W12,
            "c": c,
        }
        for i in range(NCORES)
    ]


def kernel(x, W1, bias1, W2, bias2):
    from concourse.bass_utils import run_bass_kernel_spmd

    nc = get_nc()
    in_maps = make_in_maps(x, W1, bias1, W2, bias2)
    res = run_bass_kernel_spmd(nc, in_maps, list(range(NCORES)))
    return np.concatenate([res.results[i]["out"] for i in range(NCORES)], axis=0)


# revision 70
# speedup vs baseline: 1.1541x; 1.0223x over previous
"""Bass/Tile TRN2 kernel for nn_AttentionHead (B=64, N=1024, d=512), 8-core data parallel.

Math (per batch):
    proj  = x @ W1 + b1                      [N, 2d]
    S     = proj @ relu(proj).T / sqrt(2d)   [N, N]
    P     = softmax(S, axis=-1)
    F     = P @ proj                         [N, 2d]
    out   = relu(F @ W2 + b2)                [N, d]

Kernel dataflow (transposed-score formulation, value path folded through W12):
    xT    = x.T (DMA transpose)                                 [d, N]
    projT = W1.T @ xT + b1; keyT = relu(projT)   (stored fp8)   [2d, N]
    St[m,n] = sum_e keyT[e,m] projT[e,n]   (fp8 DoubleRow)      [m, n]
    Et    = exp(St * scale);  r[n] = sum_m Et[m,n]  (ones matmul)
    V2[m,t] = c[t] + sum_d x[m,d] W12[d,t],  W12 = W1@W2 (host) [m, t]
    Z[n,t]  = sum_m Et[m,n] V2[m,t]  ( = P_unnorm @ (x@W1W2) + r*c )
    out   = relu(Z[n,t]/r[n]),  c = b1 @ W2 + b2 (host)
Precision: the logit path (projT/keyT storage + the scores matmul) runs in
fp8 e4m3 with DoubleRow double-pumping (256-deep contraction per PE pass);
fc1 and the whole value path stay bf16 so fp8 noise is not compounded and
never touches values directly — softmax weight noise attenuates. W1/b1 are
pre-scaled by 8 on the host so W1 sits in e4m3 normal range; the resulting
x64 on the scores is undone in the exp scale. The r*c rank-1 bias term is
obtained for free by adding c into V2 before the Z matmuls.
A post-compile pass (_dedup_ldweights) elides back-to-back repeated
LDWEIGHTS (mostly cosmetic: LDW overlaps matmul on TRN2).
"""

import numpy as np

B, N, D = 64, 1024, 512
E = 2 * D
NCORES = 8
BPC = B // NCORES
P = 128
MG = N // P  # 8 token groups
DG = D // P  # 4 d groups
EG = E // P  # 8 e groups
NJ = N // 512  # 2 free-dim chunks
SCALE = float(1.0 / np.sqrt(2.0 * D))

_CACHE = {}


def _dedup_ldweights(nc):
    """Delete redundant InstLdweights: consecutive PE weight-loads of the same
    SBUF region keep the PE array's stationary operand, so the repeat load is a
    no-op costing ~107ns. Only sync-free LDWs are removed (waits/updates were
    already hoisted by bacc's move_matmul_waits_to_ldweights)."""
    import concourse.mybir as mybir

    removed = 0
    for bb in nc.m.functions[0].blocks:
        last_key = None
        keep = []
        for inst in bb.instructions:
            if str(getattr(inst, "engine", "")) != "EngineType.PE":
                keep.append(inst)
                continue
            if isinstance(inst, mybir.InstLdweights):
                ap = inst.ins[0]
                key = (
                    getattr(ap, "memref", None),
                    getattr(ap, "offset", None),
                    str(getattr(ap, "ap", None)),
                    str(getattr(ap, "dtype", None)),
                    str(getattr(inst, "tile_position", None)),
                    str(getattr(inst, "is_transpose", None)),
                )
                si = inst.sync_info
                sync_free = si is None or (not si.on_wait and not si.on_update)
                if key == last_key and sync_free:
                    removed += 1
                    continue
                last_key = key
            keep.append(inst)
        bb.instructions[:] = keep
    return removed


def _build(bpc=BPC):
    import concourse.mybir as mybir
    import concourse.tile as tile
    from concourse import bacc
    from contextlib import ExitStack

    BF = mybir.dt.bfloat16
    F8 = mybir.dt.float8e4
    F32 = mybir.dt.float32
    AF = mybir.ActivationFunctionType
    ALU = mybir.AluOpType
    DR = mybir.MatmulPerfMode.DoubleRow

    nc = bacc.Bacc("TRN2", target_bir_lowering=False, debug=False, num_devices=NCORES)
    x_d = nc.dram_tensor("x", [bpc, N, D], F32, kind="ExternalInput").ap()
    w1_d = nc.dram_tensor("W1", [D, E], BF, kind="ExternalInput").ap()  # bf16 (host)
    b1_d = nc.dram_tensor("bias1", [E], F32, kind="ExternalInput").ap()
    w12_d = nc.dram_tensor("W12", [D, D], BF, kind="ExternalInput").ap()  # W1@W2 bf16 (host)
    c_d = nc.dram_tensor("c", [D], BF, kind="ExternalInput").ap()  # b1@W2 + b2 (host)
    out_d = nc.dram_tensor("out", [bpc, N, D], F32, kind="ExternalOutput").ap()

    with tile.TileContext(nc) as tc, ExitStack() as ctx:
        stage = ctx.enter_context(tc.tile_pool(name="stage", bufs=2))
        consts = ctx.enter_context(tc.tile_pool(name="consts", bufs=1))
        xbf_p = ctx.enter_context(tc.tile_pool(name="xbf", bufs=2))
        xt_p = ctx.enter_context(tc.tile_pool(name="xt", bufs=2))
        projT_p = ctx.enter_context(tc.tile_pool(name="projT", bufs=2))
        keyT_p = ctx.enter_context(tc.tile_pool(name="keyT", bufs=2))
        e_p = ctx.enter_context(tc.tile_pool(name="Et", bufs=2))
        v2_p = ctx.enter_context(tc.tile_pool(name="V2", bufs=2))
        outp = ctx.enter_context(tc.tile_pool(name="outp", bufs=2))
        small = ctx.enter_context(tc.tile_pool(name="small", bufs=2))
        dram = ctx.enter_context(tc.tile_pool(name="dram", bufs=2, space="DRAM"))
        ps = ctx.enter_context(tc.tile_pool(name="ps", bufs=6, space="PSUM"))
        psC = ctx.enter_context(tc.tile_pool(name="psC", bufs=2, space="PSUM"))

        # ---------------- x prep: load + cast + xbar transpose ----------------
        # PREP(b) is emitted during body b-1 (software pipelining) so the
        # vector casts sit ahead of body b-1's epilogue in the vector queue
        # and xT(b) is ready the moment fc1(b) can issue on PE.
        def prep(b):
            x_st = stage.tile([P, MG, D], F32, tag="stage")
            x_bf = xbf_p.tile([P, MG, D], BF)
            xT = xt_p.tile([P, DG, N], BF)
            if b == 0:
                # startup latency chain: alternate load queues (sync/gpsimd —
                # safe only before any out-stores exist) so each transpose can
                # chase its chunk instead of queueing behind the whole load
                for mg in range(MG):
                    ld = nc.sync if mg % 2 == 0 else nc.gpsimd
                    ld.dma_start(
                        out=x_st[:, mg, :], in_=x_d[0][mg * P : (mg + 1) * P, :]
                    )
                    nc.vector.tensor_copy(x_bf[:, mg, :], x_st[:, mg, :])
                    nc.sync.dma_start(
                        out=xT[:, :, mg * P : (mg + 1) * P],
                        in_=x_bf[:, mg, :],
                        transpose=True,
                    )
                return xT
            nc.sync.dma_start(
                out=x_st[:], in_=x_d[b].rearrange("(g p) d -> p g d", p=P)
            )
            for mg in range(MG):
                nc.vector.tensor_copy(x_bf[:, mg, :], x_st[:, mg, :])
                nc.sync.dma_start(
                    out=xT[:, :, mg * P : (mg + 1) * P],
                    in_=x_bf[:, mg, :],
                    transpose=True,
                )
            return xT

        xT = prep(0)

        # ---------------- constants / weights ----------------
        # W1 arrives pre-scaled by 8 (host) so it sits in e4m3 normal range;
        # the x8 on projT/keyT (x64 on scores) is undone in the exp scale.
        w1_bf = consts.tile([P, DG, E], BF)
        nc.scalar.dma_start(out=w1_bf[:], in_=w1_d.rearrange("(dg p) e -> p dg e", p=P))

        w12_bf = consts.tile([P, DG, D], BF)
        nc.scalar.dma_start(
            out=w12_bf[:], in_=w12_d.rearrange("(dg p) t -> p dg t", p=P)
        )

        b1t = consts.tile([P, EG], F32)
        nc.scalar.dma_start(out=b1t[:], in_=b1_d.rearrange("(g p) -> p g", p=P))
        ones_sq = consts.tile([P, P], BF)
        nc.vector.memset(ones_sq[:], 1.0)

        # c = b1@W2 + b2 (host, bf16), broadcast to all partitions. Adding c[t]
        # to every V2 row makes the fc2 psum Z + r[n]*c[t] (since sum_m Et = r),
        # so the epilogue is just out = relu(psum/r).
        import concourse.bass as bass_mod

        c_bcast = consts.tile([P, D], BF)
        c_src = c_d.rearrange("(o t) -> o t", o=1)
        c_bcast_ap = bass_mod.AP(
            tensor=c_src.tensor,
            offset=c_src.offset,
            ap=[[0, P], c_src.ap[1]],
        )
        nc.scalar.dma_start(out=c_bcast[:], in_=c_bcast_ap)

        # ---------------- per-batch pipeline ----------------
        for b in range(bpc):
            # fc1 (bf16): projT = 8*(W1.T @ xT) + 8*b1 ; keyT = relu(projT)
            # (projT/keyT are stored fp8 for the DoubleRow scores matmul; fc1
            #  itself stays bf16 so the fp8 quantization error isn't compounded)
            projT = projT_p.tile([P, EG, N], F8)
            keyT = keyT_p.tile([P, EG, N], F8)
            for eg in range(EG):
                pf = [ps.tile([P, 512], F32, tag="ps", name=f"pf{eg}_{j}") for j in range(NJ)]
                for dg in range(DG):
                    for nj in range(NJ):
                        nc.tensor.matmul(
                            pf[nj][:],
                            w1_bf[:, dg, eg * P : (eg + 1) * P],
                            xT[:, dg, nj * 512 : (nj + 1) * 512],
                            start=(dg == 0), stop=(dg == DG - 1),
                        )
                for nj in range(NJ):
                    nsl = slice(nj * 512, (nj + 1) * 512)
                    nc.scalar.activation(
                        projT[:, eg, nsl], pf[nj][:], AF.Identity,
                        bias=b1t[:, eg : eg + 1], scale=1.0,
                    )
                    nc.vector.tensor_scalar(
                        out=keyT[:, eg, nsl], in0=pf[nj][:],
                        scalar1=b1t[:, eg : eg + 1], scalar2=0.0,
                        op0=ALU.add, op1=ALU.max,
                    )

            # next batch's x prep: emitted after fc1 so its vector casts run
            # after this batch's keyT drains; all DMAs overlap this compute
            if b < bpc - 1:
                xT_next = prep(b + 1)

            # V2[m,t] = c[t] + sum_d x[m,d] W12[d,t]  (independent of scores;
            # placed here to cover the projT/keyT PSUM-drain latency on PE)
            V2 = v2_p.tile([P, MG, D], BF)
            for mg in range(MG):
                pv = ps.tile([P, 512], F32, tag="ps", name=f"pv{mg}")
                for dg in range(DG):
                    nc.tensor.matmul(
                        pv[:],
                        xT[:, dg, mg * P : (mg + 1) * P],
                        w12_bf[:, dg, :],
                        start=(dg == 0), stop=(dg == DG - 1),
                    )
                nc.vector.tensor_add(V2[:, mg, :], pv[:], c_bcast[:])

            # St[m,n] = 64*sum_e keyT[e,m]*projT[e,n];  Et = exp(St*SCALE/64)
            Et = e_p.tile([P, MG, N], BF)
            for mg in range(MG):
                pst = [ps.tile([P, 512], F32, tag="ps", name=f"pst{mg}_{j}") for j in range(NJ)]
                for egp in range(EG // 2):
                    for nj in range(NJ):
                        nc.tensor.matmul(
                            pst[nj][:],
                            keyT[:, 2 * egp : 2 * egp + 2, mg * P : (mg + 1) * P],
                            projT[:, 2 * egp : 2 * egp + 2, nj * 512 : (nj + 1) * 512],
                            start=(egp == 0), stop=(egp == EG // 2 - 1),
                            perf_mode=DR,
                        )
                for nj in range(NJ):
                    nc.scalar.activation(
                        Et[:, mg, nj * 512 : (nj + 1) * 512], pst[nj][:], AF.Exp,
                        bias=0.0, scale=SCALE / 64.0,
                    )

            # rowsum r[n] = sum_m Et[m,n] (all-ones stationary; any psum row = sum)
            r_f32 = small.tile([1, N], F32)
            pr = [ps.tile([P, 512], F32, tag="ps", name=f"pr{j}") for j in range(NJ)]
            for mg in range(MG):
                for nj in range(NJ):
                    nc.tensor.matmul(
                        pr[nj][:], ones_sq[:], Et[:, mg, nj * 512 : (nj + 1) * 512],
                        start=(mg == 0), stop=(mg == MG - 1),
                    )
            for nj in range(NJ):
                nsl = slice(nj * 512, (nj + 1) * 512)
                nc.vector.tensor_copy(r_f32[:, nsl], pr[nj][0:1, :])

            # 1/r in [n-partition, 1] layout (bounce through DRAM to transpose).
            # On the scalar queue: keeps the sync queue free for the next
            # batch's x load + transposes.
            r_dram = dram.tile([N], F32)
            nc.scalar.dma_start(out=r_dram.rearrange("(o n) -> o n", o=1), in_=r_f32[:1, :])
            rT = small.tile([P, MG], F32)
            nc.scalar.dma_start(out=rT[:], in_=r_dram.rearrange("(j p) -> p j", p=P))
            rinv = small.tile([P, MG], F32)
            nc.vector.reciprocal(rinv[:], rT[:])

            # Z[n,t] = sum_m Et[m,n] V2[m,t] + r[n]*c[t];  out = relu(Z/r)
            o_t = outp.tile([P, MG, D], F32)
            for ng in range(MG):
                po = psC.tile([P, D], F32, tag="psC")
                for mg in range(MG):
                    nc.tensor.matmul(
                        po[:],
                        Et[:, mg, ng * P : (ng + 1) * P],
                        V2[:, mg, :],
                        start=(mg == 0), stop=(mg == MG - 1),
                    )
                osl = o_t[:, ng, :]
                nc.scalar.activation(
                    osl, po[:], AF.Relu, bias=0.0, scale=rinv[:, ng : ng + 1]
                )
                if b == bpc - 1:
                    eng = nc.gpsimd if ng % 2 == 0 else nc.sync
                    eng.dma_start(
                        out=out_d[b][ng * P : (ng + 1) * P, :], in_=osl
                    )
            if b < bpc - 1:
                nc.gpsimd.dma_start(
                    out=out_d[b].rearrange("(g p) t -> p g t", p=P), in_=o_t[:]
                )
                xT = xT_next

    nc.compile()
    _dedup_ldweights(nc)
    return nc


def get_nc(bpc=BPC):
    if bpc not in _CACHE:
        _CACHE[bpc] = _build(bpc)
    return _CACHE[bpc]


def make_in_maps(x, W1, bias1, W2, bias2):
    x = np.ascontiguousarray(x, dtype=np.float32)
    W1 = np.asarray(W1, dtype=np.float32)
    bias1 = np.asarray(bias1, dtype=np.float32)
    W2 = np.asarray(W2, dtype=np.float32)
    bias2 = np.asarray(bias2, dtype=np.float32)
    c = (bias1 @ W2 + bias2).astype(np.float32)
    W12 = (W1 @ W2).astype(np.float32)
    # logit path runs in fp8: pre-scale W1/b1 by 8 so W1 lands in e4m3
    # normal range; the kernel divides the scores by 64 in the exp scale.
    # W1/W12 ship as bf16 so the kernel needs no weight casts.
    import ml_dtypes

    W1s = (8.0 * W1).astype(ml_dtypes.bfloat16)
    b1s = (8.0 * bias1).astype(np.float32)
    W12 = W12.astype(ml_dtypes.bfloat16)
    c = c.astype(ml_dtypes.bfloat16)
    return [
        {
            "x": x[i * BPC : (i + 1) * BPC],
            "W1": W1s,
            "bias1": b1s,
            "W12": W12,
            "c": c,
        }
        for i in range(NCORES)
    ]


def kernel(x, W1, bias1, W2, bias2):
    from concourse.bass_utils import run_bass_kernel_spmd

    nc = get_nc()
    in_maps = make_in_maps(x, W1, bias1, W2, bias2)
    res = run_bass_kernel_spmd(nc, in_maps, list(range(NCORES)))
    return np.concatenate([res.results[i]["out"] for i in range(NCORES)], axis=0)


# revision 76
# speedup vs baseline: 1.2198x; 1.0569x over previous
"""Bass/Tile TRN2 kernel for nn_AttentionHead (B=64, N=1024, d=512), 8-core data parallel.

Math (per batch):
    proj  = x @ W1 + b1                      [N, 2d]
    S     = proj @ relu(proj).T / sqrt(2d)   [N, N]
    P     = softmax(S, axis=-1)
    F     = P @ proj                         [N, 2d]
    out   = relu(F @ W2 + b2)                [N, d]

Kernel dataflow (transposed-score formulation, value path folded through W12):
    xT    = x.T (DMA transpose)                                 [d, N]
    projT = W1.T @ xT + b1; keyT = relu(projT)   (stored fp8)   [2d, N]
    St[m,n] = sum_e keyT[e,m] projT[e,n]   (fp8 DoubleRow)      [m, n]
    Et    = exp(St * scale);  r[n] = sum_m Et[m,n]  (ones matmul)
    V2[m,t] = c[t] + sum_d x[m,d] W12[d,t],  W12 = W1@W2 (host) [m, t]
    Z[n,t]  = sum_m Et[m,n] V2[m,t]  ( = P_unnorm @ (x@W1W2) + r*c )
    out   = relu(Z[n,t]/r[n]),  c = b1 @ W2 + b2 (host)
Precision: the logit path (projT/keyT storage + the scores matmul) runs in
fp8 e4m3 with DoubleRow double-pumping (256-deep contraction per PE pass);
fc1 and the whole value path stay bf16 so fp8 noise is not compounded and
never touches values directly — softmax weight noise attenuates. W1/b1 are
pre-scaled by 8 on the host so W1 sits in e4m3 normal range; the resulting
x64 on the scores is undone in the exp scale. The r*c rank-1 bias term is
obtained for free by adding c into V2 before the Z matmuls.
A post-compile pass (_dedup_ldweights) elides back-to-back repeated
LDWEIGHTS (mostly cosmetic: LDW overlaps matmul on TRN2).
"""

import numpy as np

B, N, D = 64, 1024, 512
E = 2 * D
NCORES = 8
BPC = B // NCORES
P = 128
MG = N // P  # 8 token groups
DG = D // P  # 4 d groups
EG = E // P  # 8 e groups
NJ = N // 512  # 2 free-dim chunks
SCALE = float(1.0 / np.sqrt(2.0 * D))

_CACHE = {}


def _dedup_ldweights(nc):
    """Delete redundant InstLdweights: consecutive PE weight-loads of the same
    SBUF region keep the PE array's stationary operand, so the repeat load is a
    no-op costing ~107ns. Only sync-free LDWs are removed (waits/updates were
    already hoisted by bacc's move_matmul_waits_to_ldweights)."""
    import concourse.mybir as mybir

    removed = 0
    for bb in nc.m.functions[0].blocks:
        last_key = None
        keep = []
        for inst in bb.instructions:
            if str(getattr(inst, "engine", "")) != "EngineType.PE":
                keep.append(inst)
                continue
            if isinstance(inst, mybir.InstLdweights):
                ap = inst.ins[0]
                key = (
                    getattr(ap, "memref", None),
                    getattr(ap, "offset", None),
                    str(getattr(ap, "ap", None)),
                    str(getattr(ap, "dtype", None)),
                    str(getattr(inst, "tile_position", None)),
                    str(getattr(inst, "is_transpose", None)),
                )
                si = inst.sync_info
                sync_free = si is None or (not si.on_wait and not si.on_update)
                if key == last_key and sync_free:
                    removed += 1
                    continue
                last_key = key
            keep.append(inst)
        bb.instructions[:] = keep
    return removed


def _build(bpc=BPC):
    import concourse.mybir as mybir
    import concourse.tile as tile
    from concourse import bacc
    from contextlib import ExitStack

    BF = mybir.dt.bfloat16
    F8 = mybir.dt.float8e4
    F32 = mybir.dt.float32
    AF = mybir.ActivationFunctionType
    ALU = mybir.AluOpType
    DR = mybir.MatmulPerfMode.DoubleRow

    nc = bacc.Bacc("TRN2", target_bir_lowering=False, debug=False, num_devices=NCORES)
    x_d = nc.dram_tensor("x", [bpc, N, D], F32, kind="ExternalInput").ap()
    w1_d = nc.dram_tensor("W1", [D, E], BF, kind="ExternalInput").ap()  # bf16 (host)
    b1_d = nc.dram_tensor("bias1", [E], F32, kind="ExternalInput").ap()
    w12_d = nc.dram_tensor("W12", [D, D], BF, kind="ExternalInput").ap()  # W1@W2 bf16 (host)
    c_d = nc.dram_tensor("c", [D], BF, kind="ExternalInput").ap()  # b1@W2 + b2 (host)
    out_d = nc.dram_tensor("out", [bpc, N, D], F32, kind="ExternalOutput").ap()

    with tile.TileContext(nc) as tc, ExitStack() as ctx:
        stage = ctx.enter_context(tc.tile_pool(name="stage", bufs=2))
        consts = ctx.enter_context(tc.tile_pool(name="consts", bufs=1))
        xbf_p = ctx.enter_context(tc.tile_pool(name="xbf", bufs=2))
        xt_p = ctx.enter_context(tc.tile_pool(name="xt", bufs=2))
        projT_p = ctx.enter_context(tc.tile_pool(name="projT", bufs=2))
        keyT_p = ctx.enter_context(tc.tile_pool(name="keyT", bufs=2))
        e_p = ctx.enter_context(tc.tile_pool(name="Et", bufs=2))
        e8_p = ctx.enter_context(tc.tile_pool(name="Et8", bufs=1))
        v2_p = ctx.enter_context(tc.tile_pool(name="V2", bufs=2))
        outp = ctx.enter_context(tc.tile_pool(name="outp", bufs=2))
        small = ctx.enter_context(tc.tile_pool(name="small", bufs=2))
        dram = ctx.enter_context(tc.tile_pool(name="dram", bufs=2, space="DRAM"))
        ps = ctx.enter_context(tc.tile_pool(name="ps", bufs=6, space="PSUM"))
        psC = ctx.enter_context(tc.tile_pool(name="psC", bufs=2, space="PSUM"))

        # ---------------- x prep: load + cast + xbar transpose ----------------
        # PREP(b) is emitted during body b-1 (software pipelining) so the
        # vector casts sit ahead of body b-1's epilogue in the vector queue
        # and xT(b) is ready the moment fc1(b) can issue on PE.
        def prep(b):
            x_st = stage.tile([P, MG, D], F32, tag="stage")
            x_bf = xbf_p.tile([P, MG, D], BF)
            xT = xt_p.tile([P, DG, N], BF)
            if b == 0:
                # startup latency chain: alternate load queues (sync/gpsimd —
                # safe only before any out-stores exist) so each transpose can
                # chase its chunk instead of queueing behind the whole load
                for mg in range(MG):
                    ld = nc.sync if mg % 2 == 0 else nc.gpsimd
                    ld.dma_start(
                        out=x_st[:, mg, :], in_=x_d[0][mg * P : (mg + 1) * P, :]
                    )
                    nc.vector.tensor_copy(x_bf[:, mg, :], x_st[:, mg, :])
                    nc.sync.dma_start(
                        out=xT[:, :, mg * P : (mg + 1) * P],
                        in_=x_bf[:, mg, :],
                        transpose=True,
                    )
                return xT
            nc.sync.dma_start(
                out=x_st[:], in_=x_d[b].rearrange("(g p) d -> p g d", p=P)
            )
            return x_st, x_bf, xT

        # cast + transpose half of prep for b>=1, emitted after the exp stage:
        # the casts run on scalar BEHIND this batch's projT/exp drains, and
        # stay out of the in-order vector queue entirely (where they
        # transitively stalled PE at pipeline-fill time)
        def prep_xt(x_st, x_bf, xT):
            for mg in range(MG):
                nc.scalar.copy(x_bf[:, mg, :], x_st[:, mg, :])
                nc.sync.dma_start(
                    out=xT[:, :, mg * P : (mg + 1) * P],
                    in_=x_bf[:, mg, :],
                    transpose=True,
                )
            return xT

        xT = prep(0)

        # ---------------- constants / weights ----------------
        # W1 arrives pre-scaled by 8 (host) so it sits in e4m3 normal range;
        # the x8 on projT/keyT (x64 on scores) is undone in the exp scale.
        w1_bf = consts.tile([P, DG, E], BF)
        nc.scalar.dma_start(out=w1_bf[:], in_=w1_d.rearrange("(dg p) e -> p dg e", p=P))

        w12_bf = consts.tile([P, DG, D], BF)
        nc.scalar.dma_start(
            out=w12_bf[:], in_=w12_d.rearrange("(dg p) t -> p dg t", p=P)
        )

        b1t = consts.tile([P, EG], F32)
        nc.scalar.dma_start(out=b1t[:], in_=b1_d.rearrange("(g p) -> p g", p=P))
        ones_f8 = consts.tile([P, 2, P], F8)
        nc.vector.memset(ones_f8[:], 1.0)

        # c = b1@W2 + b2 (host, bf16), broadcast to all partitions. Adding c[t]
        # to every V2 row makes the fc2 psum Z + r[n]*c[t] (since sum_m Et = r),
        # so the epilogue is just out = relu(psum/r).
        import concourse.bass as bass_mod

        c_bcast = consts.tile([P, D], BF)
        c_src = c_d.rearrange("(o t) -> o t", o=1)
        c_bcast_ap = bass_mod.AP(
            tensor=c_src.tensor,
            offset=c_src.offset,
            ap=[[0, P], c_src.ap[1]],
        )
        nc.scalar.dma_start(out=c_bcast[:], in_=c_bcast_ap)

        # ---------------- per-batch pipeline ----------------
        for b in range(bpc):
            # fc1 (bf16): projT = 8*(W1.T @ xT) + 8*b1 ; keyT = relu(projT)
            # (projT/keyT are stored fp8 for the DoubleRow scores matmul; fc1
            #  itself stays bf16 so the fp8 quantization error isn't compounded)
            projT = projT_p.tile([P, EG, N], F8)
            keyT = keyT_p.tile([P, EG, N], F8)
            for eg in range(EG):
                pf = [ps.tile([P, 512], F32, tag="ps", name=f"pf{eg}_{j}") for j in range(NJ)]
                for dg in range(DG):
                    for nj in range(NJ):
                        nc.tensor.matmul(
                            pf[nj][:],
                            w1_bf[:, dg, eg * P : (eg + 1) * P],
                            xT[:, dg, nj * 512 : (nj + 1) * 512],
                            start=(dg == 0), stop=(dg == DG - 1),
                        )
                for nj in range(NJ):
                    nsl = slice(nj * 512, (nj + 1) * 512)
                    nc.scalar.activation(
                        projT[:, eg, nsl], pf[nj][:], AF.Identity,
                        bias=b1t[:, eg : eg + 1], scale=1.0,
                    )
                    nc.vector.tensor_scalar(
                        out=keyT[:, eg, nsl], in0=pf[nj][:],
                        scalar1=b1t[:, eg : eg + 1], scalar2=0.0,
                        op0=ALU.add, op1=ALU.max,
                    )

            # next batch's x load starts now (overlaps this batch's compute);
            # its cast+transpose half is emitted after the exp stage below
            if b < bpc - 1:
                prep_next = prep(b + 1)

            # V2[m,t] = c[t] + sum_d x[m,d] W12[d,t]  (independent of scores;
            # placed here to cover the projT/keyT PSUM-drain latency on PE)
            V2 = v2_p.tile([P, MG, D], BF)
            for mg in range(MG):
                pv = ps.tile([P, 512], F32, tag="ps", name=f"pv{mg}")
                for dg in range(DG):
                    nc.tensor.matmul(
                        pv[:],
                        xT[:, dg, mg * P : (mg + 1) * P],
                        w12_bf[:, dg, :],
                        start=(dg == 0), stop=(dg == DG - 1),
                    )
                nc.vector.tensor_add(V2[:, mg, :], pv[:], c_bcast[:])

            # St[m,n] = 64*sum_e keyT[e,m]*projT[e,n];  Et = exp(St*SCALE/64)
            # Et8 = Et/16 in fp8 feeds the DoubleRow rowsum (the /16 keeps the
            # softmax diagonal, up to ~1200, inside e4m3 range; r tolerates
            # fp8 noise since it is a positive sum)
            Et = e_p.tile([P, MG, N], BF)
            Et8 = e8_p.tile([P, MG, N], F8)
            for mg in range(MG):
                pst = [ps.tile([P, 512], F32, tag="ps", name=f"pst{mg}_{j}") for j in range(NJ)]
                for egp in range(EG // 2):
                    for nj in range(NJ):
                        nc.tensor.matmul(
                            pst[nj][:],
                            keyT[:, 2 * egp : 2 * egp + 2, mg * P : (mg + 1) * P],
                            projT[:, 2 * egp : 2 * egp + 2, nj * 512 : (nj + 1) * 512],
                            start=(egp == 0), stop=(egp == EG // 2 - 1),
                            perf_mode=DR,
                        )
                for nj in range(NJ):
                    nsl = slice(nj * 512, (nj + 1) * 512)
                    nc.scalar.activation(
                        Et[:, mg, nsl], pst[nj][:], AF.Exp,
                        bias=0.0, scale=SCALE / 64.0,
                    )
                    nc.vector.tensor_scalar_mul(
                        Et8[:, mg, nsl], Et[:, mg, nsl], 1.0 / 16.0
                    )

            if b < bpc - 1:
                xT_next = prep_xt(*prep_next)

            # rowsum r[n] = sum_m Et[m,n] = 16*sum Et8 (fp8 DoubleRow; all-ones
            # stationary; any psum row = the sum)
            r_f32 = small.tile([1, N], F32)
            pr = [ps.tile([P, 512], F32, tag="ps", name=f"pr{j}") for j in range(NJ)]
            for mgp in range(MG // 2):
                for nj in range(NJ):
                    nc.tensor.matmul(
                        pr[nj][:], ones_f8[:],
                        Et8[:, 2 * mgp : 2 * mgp + 2, nj * 512 : (nj + 1) * 512],
                        start=(mgp == 0), stop=(mgp == MG // 2 - 1),
                        perf_mode=DR,
                    )
            for nj in range(NJ):
                nsl = slice(nj * 512, (nj + 1) * 512)
                nc.vector.tensor_scalar_mul(r_f32[:, nsl], pr[nj][0:1, :], 16.0)

            # 1/r in [n-partition, 1] layout (bounce through DRAM to transpose).
            # On the scalar queue: keeps the sync queue free for the next
            # batch's x load + transposes.
            r_dram = dram.tile([N], F32)
            nc.scalar.dma_start(out=r_dram.rearrange("(o n) -> o n", o=1), in_=r_f32[:1, :])
            rT = small.tile([P, MG], F32)
            nc.scalar.dma_start(out=rT[:], in_=r_dram.rearrange("(j p) -> p j", p=P))
            rinv = small.tile([P, MG], F32)
            nc.vector.reciprocal(rinv[:], rT[:])

            # Z[n,t] = sum_m Et[m,n] V2[m,t] + r[n]*c[t];  out = relu(Z/r)
            o_t = outp.tile([P, MG, D], F32)
            for ng in range(MG):
                po = psC.tile([P, D], F32, tag="psC")
                for mg in range(MG):
                    nc.tensor.matmul(
                        po[:],
                        Et[:, mg, ng * P : (ng + 1) * P],
                        V2[:, mg, :],
                        start=(mg == 0), stop=(mg == MG - 1),
                    )
                osl = o_t[:, ng, :]
                nc.scalar.activation(
                    osl, po[:], AF.Relu, bias=0.0, scale=rinv[:, ng : ng + 1]
                )
                if b == bpc - 1:
                    eng = nc.gpsimd if ng % 2 == 0 else nc.sync
                    eng.dma_start(
                        out=out_d[b][ng * P : (ng + 1) * P, :], in_=osl
                    )
            if b < bpc - 1:
                nc.gpsimd.dma_start(
                    out=out_d[b].rearrange("(g p) t -> p g t", p=P), in_=o_t[:]
                )
                xT = xT_next

    nc.compile()
    _dedup_ldweights(nc)
    return nc


def get_nc(bpc=BPC):
    if bpc not in _CACHE:
        _CACHE[bpc] = _build(bpc)
    return _CACHE[bpc]


def make_in_maps(x, W1, bias1, W2, bias2):
    x = np.ascontiguousarray(x, dtype=np.float32)
    W1 = np.asarray(W1, dtype=np.float32)
    bias1 = np.asarray(bias1, dtype=np.float32)
    W2 = np.asarray(W2, dtype=np.float32)
    bias2 = np.asarray(bias2, dtype=np.float32)
    c = (bias1 @ W2 + bias2).astype(np.float32)
    W12 = (W1 @ W2).astype(np.float32)
    # logit path runs in fp8: pre-scale W1/b1 by 8 so W1 lands in e4m3
    # normal range; the kernel divides the scores by 64 in the exp scale.
    # W1/W12 ship as bf16 so the kernel needs no weight casts.
    import ml_dtypes

    W1s = (8.0 * W1).astype(ml_dtypes.bfloat16)
    b1s = (8.0 * bias1).astype(np.float32)
    W12 = W12.astype(ml_dtypes.bfloat16)
    c = c.astype(ml_dtypes.bfloat16)
    return [
        {
            "x": x[i * BPC : (i + 1) * BPC],
            "W1": W1s,
            "bias1": b1s,
            "W12": W12,
            "c": c,
        }
        for i in range(NCORES)
    ]


def kernel(x, W1, bias1, W2, bias2):
    from concourse.bass_utils import run_bass_kernel_spmd

    nc = get_nc()
    in_maps = make_in_maps(x, W1, bias1, W2, bias2)
    res = run_bass_kernel_spmd(nc, in_maps, list(range(NCORES)))
    return np.concatenate([res.results[i]["out"] for i in range(NCORES)], axis=0)
